# revision 4
# baseline (speedup 1.0000x reference)
"""BertSelfAttention (rotary, 16 heads, hd=64) on 8 trn2 cores.

Sharding: data-parallel over batch (4) x tensor-parallel over heads (2 groups
of 8). Core c handles batch c//2, head-group c%2. Each core computes its
heads' QKV projection, rotary, full attention, and writes ctx^T [512, 2048];
the host transposes/concatenates into the full [4, 2048, 1024] output.

Math notes:
 - scores_ref = (rot(q)/8 . rot(k))/8 = s_raw/64 with s_raw = rot(q).rot(k).
   We fold the 1/64 into the exp activation's scale.
 - |scores| <~ 0.8, so softmax without max-subtraction is numerically safe
   and mathematically identical.
 - Denominator via a ones-column appended to V (col 64 of each head block);
   ctx^T accumulates [65, i] per head where row 64 is the softmax denom.
 - Matmuls run in float32r (full PE rate at N>=256, ~1e-4 rel precision).
"""

import numpy as np

import concourse.bass as bass
import concourse.tile as tile
from concourse import bacc, bass_utils, mybir

B, S, H = 4, 2048, 1024
NH, HD = 16, 64
NCORES = 8
HPC = NH // 2            # heads per core = 8
DG = HPC * HD            # per-core head-dim group = 512
KC = H // 128            # contraction chunks = 8
DC = DG // 128           # d chunks = 4
IBLK = 256               # phase A i-block
NIB = S // IBLK          # 8
IH = 1024                # phase B i-half
NJ = S // 128            # 16 j chunks

F32 = mybir.dt.float32
F32R = mybir.dt.float32r
EXP = mybir.ActivationFunctionType.Exp

_CACHE = {}


def _emit(nc, tc, ctx, ins, o_d):
    xt_d, wq_d, wk_d, wv_d, bq_d, bk_d, bv_d, cos_d, sin_d, ones_d = ins

    persist = ctx.enter_context(tc.tile_pool(name="persist", bufs=1))
    qt = persist.tile([128, DC, S], F32R, tag="qt")
    kt = persist.tile([128, DC, S], F32R, tag="kt")
    vaug = persist.tile([128, NJ, HPC * (HD + 1)], F32R, tag="vaug")
    bq_sb = persist.tile([128, DC], F32, tag="bq")
    bk_sb = persist.tile([128, DC], F32, tag="bk")
    bv_sb = persist.tile([1, DG], F32R, tag="bv")
    ones_sb = persist.tile([128, 128], F32R, tag="ones")

    nc.sync.dma_start(bq_sb[:], bq_d)
    nc.sync.dma_start(bk_sb[:], bk_d)
    nc.sync.dma_start(bv_sb[:], bv_d.bitcast(F32R))
    nc.sync.dma_start(ones_sb[:], ones_d.bitcast(F32R))
    ones128 = ones_sb[0:1, :]
    ones64 = ones_sb[0:1, 0:HD]

    # ---------------- Phase A: QKV projection + rotary ----------------
    with (
        tc.tile_pool(name="wpool", bufs=1) as wpool,
        tc.tile_pool(name="xpool", bufs=2) as xpool,
        tc.tile_pool(name="cpool", bufs=2) as cpool,
        tc.tile_pool(name="tpool", bufs=3) as tpool,
        tc.tile_pool(name="qkps", bufs=2, space="PSUM") as qkps,
        tc.tile_pool(name="vps", bufs=2, space="PSUM") as vps,
    ):
        wq_sb = wpool.tile([128, KC, DG], F32R, tag="wq")
        wk_sb = wpool.tile([128, KC, DG], F32R, tag="wk")
        wv_sb = wpool.tile([128, KC, DG], F32R, tag="wv")
        for w_sb, w_d in ((wq_sb, wq_d), (wk_sb, wk_d), (wv_sb, wv_d)):
            nc.sync.dma_start(
                w_sb[:], w_d.rearrange("(c p) d -> p c d", p=128).bitcast(F32R)
            )

        xt_r = xt_d.rearrange("(c p) i -> p c i", p=128).bitcast(F32R)
        for ib in range(NIB):
            isl = slice(ib * IBLK, (ib + 1) * IBLK)
            xt_sb = xpool.tile([128, KC, IBLK], F32R, tag="xt")
            nc.sync.dma_start(xt_sb[:], xt_r[:, :, isl])
            cos_sb = cpool.tile([128, IBLK], F32, tag="cos")
            sin_sb = cpool.tile([128, IBLK], F32, tag="sin")
            nc.sync.dma_start(cos_sb[:], cos_d[:, isl])
            nc.sync.dma_start(sin_sb[:], sin_d[:, isl])

            for w_sb, b_sb, out_t in ((wq_sb, bq_sb, qt), (wk_sb, bk_sb, kt)):
                for dc in range(DC):
                    ps = qkps.tile([128, IBLK], F32, tag="qk")
                    for kc in range(KC):
                        nc.tensor.matmul(
                            ps[:],
                            w_sb[:, kc, dc * 128:(dc + 1) * 128],
                            xt_sb[:, kc, :],
                            start=(kc == 0),
                            stop=(kc == KC - 1),
                        )
                    q0 = tpool.tile([128, IBLK], F32, tag="q0")
                    nc.vector.tensor_scalar_add(q0[:], ps[:], b_sb[:, dc:dc + 1])
                    t1 = tpool.tile([128, IBLK], F32, tag="t1")
                    nc.vector.tensor_mul(t1[:], q0[:], cos_sb[:])
                    t2 = tpool.tile([128, IBLK], F32, tag="t2")
                    for g in range(2):
                        for hh in range(2):
                            po_out = g * 64 + hh * 32
                            po_in = g * 64 + (1 - hh) * 32
                            nc.vector.tensor_mul(
                                t2[po_out:po_out + 32, :],
                                q0[po_in:po_in + 32, :],
                                sin_sb[po_in:po_in + 32, :],
                            )
                    nc.vector.tensor_add(out_t[:, dc, isl], t1[:], t2[:])

            for ic2 in range(IBLK // 128):
                jc = ib * (IBLK // 128) + ic2
                vp = vps.tile([128, DG], F32, tag="v")
                for kc in range(KC):
                    nc.tensor.matmul(
                        vp[:],
                        xt_sb[:, kc, ic2 * 128:(ic2 + 1) * 128],
                        wv_sb[:, kc, :],
                        start=(kc == 0),
                        stop=False,
                    )
                nc.tensor.matmul(vp[:], ones128, bv_sb[:], start=False, stop=True)
                vv = vaug[:, jc, :].rearrange("p (h c) -> p h c", h=HPC)
                nc.vector.tensor_copy(
                    vv[:, :, HD:HD + 1],
                    ones_sb[:, 0:HPC].rearrange("p (h one) -> p h one", one=1),
                )
                nc.vector.tensor_copy(
                    vv[:, :, 0:HD], vp[:].rearrange("p (h c) -> p h c", h=HPC)
                )

    # ---------------- Phase B: attention ----------------
    with (
        tc.tile_pool(name="ppool", bufs=3) as ppool,
        tc.tile_pool(name="dpool", bufs=2) as dpool,
        tc.tile_pool(name="rpool", bufs=2) as rpool,
        tc.tile_pool(name="npool", bufs=2) as npool,
        tc.tile_pool(name="sps", bufs=2, space="PSUM") as sps,
        tc.tile_pool(name="cps", bufs=2, space="PSUM") as cps,
    ):
        for h in range(HPC):
            pc = h // 2
            po = (h % 2) * 64
            for ihalf in range(2):
                ctx_ps = cps.tile([HD + 1, IH], F32, tag="ctx")
                for j in range(NJ):
                    s_ps = sps.tile([128, IH], F32, tag="s")
                    for n in range(2):
                        nc.tensor.matmul(
                            s_ps[:, n * 512:(n + 1) * 512],
                            kt[po:po + 64, pc, j * 128:(j + 1) * 128],
                            qt[po:po + 64, pc,
                               ihalf * IH + n * 512:ihalf * IH + (n + 1) * 512],
                            start=True,
                            stop=True,
                        )
                    p_sb = ppool.tile([128, IH], F32R, tag="p")
                    nc.scalar.activation(p_sb[:], s_ps[:], EXP, scale=1.0 / 64.0)
                    for n in range(2):
                        nc.tensor.matmul(
                            ctx_ps[:, n * 512:(n + 1) * 512],
                            vaug[:, j, h * (HD + 1):(h + 1) * (HD + 1)],
                            p_sb[:, n * 512:(n + 1) * 512],
                            start=(j == 0),
                            stop=(j == NJ - 1),
                        )
                den = dpool.tile([1, IH], F32R, tag="den")
                nc.vector.tensor_copy(den[:], ctx_ps[HD:HD + 1, :])
                rb_ps = sps.tile([64, IH], F32, tag="s")
                for n in range(2):
                    nc.tensor.matmul(
                        rb_ps[:, n * 512:(n + 1) * 512],
                        ones64,
                        den[:, n * 512:(n + 1) * 512],
                        start=True,
                        stop=True,
                    )
                rec = rpool.tile([64, IH], F32, tag="rec")
                nc.vector.reciprocal(rec[:], rb_ps[:])
                ctxn = npool.tile([64, IH], F32, tag="ctxn")
                nc.vector.tensor_mul(ctxn[:], ctx_ps[0:HD, :], rec[:])
                nc.sync.dma_start(
                    o_d[h * HD:(h + 1) * HD, ihalf * IH:(ihalf + 1) * IH], ctxn[:]
                )


def _build():
    if "nc" in _CACHE:
        return _CACHE["nc"]
    nc = bacc.Bacc("TRN2", target_bir_lowering=False, debug=False,
                   num_devices=NCORES)
    names_shapes = [
        ("xt", [H, S]), ("wq", [H, DG]), ("wk", [H, DG]), ("wv", [H, DG]),
        ("bq", [128, DC]), ("bk", [128, DC]), ("bv", [1, DG]),
        ("cos", [128, S]), ("sin", [128, S]), ("ones", [128, 128]),
    ]
    ins = [nc.dram_tensor(n, s, F32, kind="ExternalInput").ap()
           for n, s in names_shapes]
    o_d = nc.dram_tensor("o", [DG, S], F32, kind="ExternalOutput").ap()
    from contextlib import ExitStack
    with tile.TileContext(nc) as tc:
        with ExitStack() as ctx:
            _emit(nc, tc, ctx, ins, o_d)
    nc.compile()
    _CACHE["nc"] = nc
    return nc


def _rotary_tables():
    inv_freq = (1.0 / (10000.0 ** (np.arange(0, HD, 2, dtype=np.float32)
                                   / np.float32(HD)))).astype(np.float32)
    t = np.arange(S, dtype=np.float32)
    freqs = np.outer(t, inv_freq).astype(np.float32)       # [S, 32]
    emb = np.concatenate([freqs, freqs], axis=-1)          # [S, 64]
    cos_t = np.cos(emb).T.astype(np.float32)               # [64, S]
    sin_t = np.sin(emb).T.astype(np.float32)
    # Table is read at the *source* partitions of the rotate-half swap (the
    # two DVE inputs must share base partition), so the sign lives on the
    # second half: t2[0:32] = q[32:64] * (-sin), t2[32:64] = q[0:32] * (+sin).
    sinm = sin_t.copy()
    sinm[HD // 2:] *= -1.0
    cos2 = np.ascontiguousarray(np.concatenate([cos_t, cos_t], axis=0))
    sinm2 = np.ascontiguousarray(np.concatenate([sinm, sinm], axis=0))
    return cos2, sinm2


def _in_maps(hidden_states, Wq, bq, Wk, bk, Wv, bv):
    cos2, sinm2 = _rotary_tables()
    xts = [np.ascontiguousarray(hidden_states[b].T) for b in range(B)]
    w_slices = {}
    for g in range(2):
        dsl = slice(g * DG, (g + 1) * DG)
        w_slices[g] = dict(
            wq=np.ascontiguousarray(Wq[:, dsl]),
            wk=np.ascontiguousarray(Wk[:, dsl]),
            wv=np.ascontiguousarray(Wv[:, dsl]),
            bq=np.ascontiguousarray(bq[dsl].reshape(DC, 128).T),
            bk=np.ascontiguousarray(bk[dsl].reshape(DC, 128).T),
            bv=np.ascontiguousarray(bv[dsl].reshape(1, DG)),
        )
    onesm = np.ones((128, 128), dtype=np.float32)
    maps = []
    for c in range(NCORES):
        b, g = c // 2, c % 2
        m = {"xt": xts[b], "cos": cos2, "sin": sinm2, "ones": onesm}
        m.update(w_slices[g])
        maps.append(m)
    return maps


def run(inputs, **kw):
    inputs = {k: np.asarray(v, dtype=np.float32) for k, v in inputs.items()}
    nc = _build()
    maps = _in_maps(**inputs)
    res = bass_utils.run_bass_kernel_spmd(nc, maps, core_ids=list(range(NCORES)),
                                          **kw)
    out = np.empty((B, S, H), dtype=np.float32)
    for c in range(NCORES):
        b, g = c // 2, c % 2
        out[b, :, g * DG:(g + 1) * DG] = res.results[c]["o"].T
    return out, res


def kernel(**inputs):
    out, _ = run(inputs)
    return out


# revision 24
# speedup vs baseline: 2.2702x; 2.2702x over previous
"""BertSelfAttention (rotary, 16 heads, hd=64) on 8 trn2 cores.

Sharding: data-parallel over batch (4) x tensor-parallel over heads (2 groups
of 8). Core c handles batch c//2, head-group c%2. Each core computes its
heads' QKV projection, rotary, full attention, and writes ctx^T [512, 2048];
the host transposes/concatenates into the full [4, 2048, 1024] output.

Math notes:
 - scores_ref = (rot(q)/8 . rot(k))/8 = s_raw/64 with s_raw = rot(q).rot(k).
   The 1/64 is folded into the exp activation's scale.
 - |scores| <~ 0.8, so softmax without max-subtraction is numerically safe
   and mathematically identical.
 - rotate_half runs on the PE as a signed permutation matmul (Rsw), keeping
   the DVE to 3 full-width ops per chunk.
 - Denominator via a ones-column appended to V (col 64 of each head block);
   ctx^T accumulates [65, i] per head where row 64 is the softmax denom.
 - Matmuls in bf16 (full PE rate + FWL); rotary intermediate math f32r/fp32.
 - Phase A pass1 computes V fully + Q/K for the first S half; attention on
   i-half 0 then overlaps with pass2 (Q/K second half).
"""

import ml_dtypes
import numpy as np

import concourse.bass as bass
import concourse.tile as tile
from concourse import bacc, bass_utils, mybir

NPBF16 = ml_dtypes.bfloat16

B, S, H = 4, 2048, 1024
NH, HD = 16, 64
NCORES = 8
HPC = NH // 2            # heads per core = 8
DG = HPC * HD            # per-core head-dim group = 512
KC = H // 128            # contraction chunks = 8
DC = DG // 128           # d chunks = 4
IBLK = 512               # phase A i-block
NIB = S // IBLK          # 4
IH = 1024                # phase B i-half
NJ = S // 128            # 16 j chunks

F32 = mybir.dt.float32
F32R = mybir.dt.float32r
BF16 = mybir.dt.bfloat16
EXP = mybir.ActivationFunctionType.Exp

_CACHE = {}


def _emit(nc, tc, ctx, ins, o_d):
    (xt_d, wq_d, wk_d, wv_d, bq_d, bk_d, bv_d, cos_d, sin_d, ones_d,
     rsw_d) = ins

    persist = ctx.enter_context(tc.tile_pool(name="persist", bufs=1))
    # per-(dc, ihalf) q tiles and per-dc k tiles: the dc-pipelined passes must
    # never write a tile an earlier attention pair still reads
    qt = [[persist.tile([128, IH], BF16, tag=f"qt{i}_{l}", name=f"qt{i}_{l}")
           for l in range(2)] for i in range(DC)]
    kt = [persist.tile([128, S], BF16, tag=f"kt{i}", name=f"kt{i}")
          for i in range(DC)]
    # per-j-chunk V tiles so ctx can start as soon as the first chunks land
    vaug = [persist.tile([128, HPC * (HD + 1)], BF16, tag=f"va{j}",
                         name=f"va{j}") for j in range(NJ)]
    bq_sb = persist.tile([128, DC], F32, tag="bq")
    bk_sb = persist.tile([128, DC], F32, tag="bk")
    bv_sb = persist.tile([1, DG], BF16, tag="bv")
    ones_sb = persist.tile([128, IBLK], BF16, tag="ones")
    rsw_sb = persist.tile([128, 128], BF16, tag="rsw")

    # small/persistent inputs go on the gpsimd (SWDGE) queue so they don't
    # delay the critical wk/xt loads on the sync queue
    nc.gpsimd.dma_start(bk_sb[:], bk_d)
    nc.gpsimd.dma_start(bq_sb[:], bq_d)
    nc.gpsimd.dma_start(bv_sb[:], bv_d)
    nc.gpsimd.dma_start(ones_sb[:], ones_d)
    nc.gpsimd.dma_start(rsw_sb[:], rsw_d)
    ones128 = ones_sb[0:1, 0:128]
    onesr = ones_sb[0:1, :]

    wpool = ctx.enter_context(tc.tile_pool(name="wpool", bufs=1))
    xpool = ctx.enter_context(tc.tile_pool(name="xpool", bufs=2))
    cpool = ctx.enter_context(tc.tile_pool(name="cpool", bufs=2))
    tpool = ctx.enter_context(tc.tile_pool(name="tpool", bufs=3))
    aps = ctx.enter_context(tc.tile_pool(name="aps", bufs=2, space="PSUM"))
    ppool = ctx.enter_context(tc.tile_pool(name="ppool", bufs=6))
    capool = ctx.enter_context(tc.tile_pool(name="capool", bufs=3))
    rpool = ctx.enter_context(tc.tile_pool(name="rpool", bufs=2))
    bpool = ctx.enter_context(tc.tile_pool(name="bpool", bufs=3))
    npool = ctx.enter_context(tc.tile_pool(name="npool", bufs=3))
    sps = ctx.enter_context(tc.tile_pool(name="sps", bufs=2, space="PSUM"))
    cps = ctx.enter_context(tc.tile_pool(name="cps", bufs=1, space="PSUM"))

    wq_sb = wpool.tile([128, KC, DG], BF16, tag="wq")
    wk_sb = wpool.tile([128, KC, DG], BF16, tag="wk")
    wv_sb = wpool.tile([128, KC, DG], BF16, tag="wv")
    nc.sync.dma_start(wk_sb[:], wk_d.rearrange("(c p) d -> p c d", p=128))
    nc.gpsimd.dma_start(wq_sb[:], wq_d.rearrange("(c p) d -> p c d", p=128))
    nc.gpsimd.dma_start(wv_sb[:], wv_d.rearrange("(c p) d -> p c d", p=128))

    xt_r = xt_d.rearrange("(c p) i -> p c i", p=128)

    def qk_chunk(w_sb, b_sb, out_t, dc, xt_sb, cos_sb, sin_sb, lsl):
        ps = aps.tile([128, IBLK], F32, tag="a", name="ps")
        for kc in range(KC):
            nc.tensor.matmul(
                ps[:], w_sb[:, kc, dc * 128:(dc + 1) * 128], xt_sb[:, kc, :],
                start=(kc == 0), stop=(kc == KC - 1),
            )
        q0 = tpool.tile([128, IBLK], BF16, tag="q0")
        nc.vector.tensor_scalar_add(q0[:], ps[:], b_sb[:, dc:dc + 1])
        t2ps = aps.tile([128, IBLK], F32, tag="a", name="t2ps")
        nc.tensor.matmul(t2ps[:], rsw_sb[:], q0[:], start=True, stop=True)
        m1 = tpool.tile([128, IBLK], F32R, tag="m1")
        nc.vector.tensor_mul(m1[:], q0[:], cos_sb[:])
        t2s = tpool.tile([128, IBLK], F32R, tag="t2s")
        nc.vector.tensor_mul(t2s[:], t2ps[:].bitcast(F32R), sin_sb[:])
        nc.vector.tensor_add(out_t[:, lsl], m1[:], t2s[:])

    def qk_pass_thunks(dc, with_v=False):
        thunks = []
        for ib in range(NIB):
            isl = slice(ib * IBLK, (ib + 1) * IBLK)
            ihalf = (ib * IBLK) // IH
            lsl = slice(ib * IBLK - ihalf * IH, (ib + 1) * IBLK - ihalf * IH)

            def load_and_k(ib=ib, isl=isl):
                xt_sb = xpool.tile([128, KC, IBLK], BF16, tag="xt",
                                   name="xt_sb")
                nc.sync.dma_start(xt_sb[:], xt_r[:, :, isl])
                cos_sb = cpool.tile([128, IBLK], BF16, tag="cos",
                                    name="cos_sb")
                sin_sb = cpool.tile([128, IBLK], F32R, tag="sin",
                                    name="sin_sb")
                nc.sync.dma_start(cos_sb[:], cos_d[:, isl])
                nc.sync.dma_start(sin_sb[:], sin_d[:, isl].bitcast(F32R))
                qk_chunk(wk_sb, bk_sb, kt[dc], dc, xt_sb, cos_sb, sin_sb, isl)
                return xt_sb, cos_sb, sin_sb

            def do_q(state, ihalf=ihalf, lsl=lsl):
                xt_sb, cos_sb, sin_sb = state
                qk_chunk(wq_sb, bq_sb, qt[dc][ihalf], dc, xt_sb, cos_sb,
                         sin_sb, lsl)
                return state

            def do_v(state, ib=ib):
                xt_sb, _, _ = state
                for ic2 in range(IBLK // 128):
                    jc = ib * (IBLK // 128) + ic2
                    vp = aps.tile([128, DG], F32, tag="a", name="vp")
                    for kc in range(KC):
                        nc.tensor.matmul(
                            vp[:],
                            xt_sb[:, kc, ic2 * 128:(ic2 + 1) * 128],
                            wv_sb[:, kc, :],
                            start=(kc == 0), stop=False,
                        )
                    nc.tensor.matmul(vp[:], ones128, bv_sb[:],
                                     start=False, stop=True)
                    vv = vaug[jc][:].rearrange("p (h c) -> p h c", h=HPC)
                    nc.vector.tensor_copy(
                        vv[:, :, HD:HD + 1],
                        ones_sb[:, 0:HPC].rearrange("p (h one) -> p h one",
                                                    one=1),
                    )
                    nc.vector.tensor_copy(
                        vv[:, :, 0:HD],
                        vp[:].rearrange("p (h c) -> p h c", h=HPC),
                    )
                return state

            steps = [load_and_k, do_q] + ([do_v] if with_v else [])
            thunks.append(steps)
        # flatten into sequential closures sharing per-ib state
        out = []
        for steps in thunks:
            state_box = {}

            def make(fn, box=state_box, first=(len(out) >= 0)):
                def run():
                    if "s" not in box:
                        box["s"] = fn()
                    else:
                        box["s"] = fn(box["s"])
                return run

            for fn in steps:
                out.append(make(fn))
        return out

    class Unit:
        def __init__(self, h, ihalf):
            self.h, self.ihalf = h, ihalf
            self.pc, self.po = h // 2, (h % 2) * 64
            self.ctx_ps = None
            self.p_tiles = [None] * NJ

        def scores(self, j):
            s_ps = sps.tile([128, IH], F32, tag="s", name="s_ps")
            qth = qt[self.pc][self.ihalf]
            for n in range(2):
                nc.tensor.matmul(
                    s_ps[:, n * 512:(n + 1) * 512],
                    kt[self.pc][self.po:self.po + 64, j * 128:(j + 1) * 128],
                    qth[self.po:self.po + 64, n * 512:(n + 1) * 512],
                    start=True, stop=True,
                )
            p_sb = ppool.tile([128, IH], BF16, tag="p", name="p_sb")
            nc.scalar.activation(p_sb[:], s_ps[:], EXP, scale=1.0 / 64.0)
            self.p_tiles[j] = p_sb

        def ctx_acc(self, j):
            if self.ctx_ps is None:
                self.ctx_ps = cps.tile([HD + 1, IH], F32, tag="ctx",
                                       name="ctx_ps")
            p_sb = self.p_tiles[j]
            for n in range(2):
                nc.tensor.matmul(
                    self.ctx_ps[:, n * 512:(n + 1) * 512],
                    vaug[j][:, self.h * (HD + 1):(self.h + 1) * (HD + 1)],
                    p_sb[:, n * 512:(n + 1) * 512],
                    start=(j == 0), stop=(j == NJ - 1),
                )
            self.p_tiles[j] = None

        def tail(self):
            h, ihalf, ctx_ps = self.h, self.ihalf, self.ctx_ps
            ca = capool.tile([HD + 1, IH], F32, tag="ca")
            nc.vector.tensor_copy(ca[:], ctx_ps[:])
            den0 = rpool.tile([1, IH], F32, tag="den0")
            nc.vector.tensor_copy(den0[:], ca[HD:HD + 1, :])
            rec1 = rpool.tile([1, IH], F32, tag="rec1")
            nc.vector.reciprocal_approx_fast(rec1[:], den0[:])
            rbc = bpool.tile([HD, IH], F32, tag="rbc")
            nc.gpsimd.partition_broadcast(rbc[:], rec1[:], channels=HD)
            ctxn = npool.tile([HD, IH], F32, tag="ctxn")
            nc.vector.tensor_mul(ctxn[:], ca[0:HD, :], rbc[:])
            nc.sync.dma_start(
                o_d[h * HD:(h + 1) * HD, ihalf * IH:(ihalf + 1) * IH],
                ctxn[:],
            )

    LAG = 2

    def attn_pair(pair, fillers):
        units = [Unit(h, l) for h in (2 * pair, 2 * pair + 1)
                 for l in range(2)]
        total = len(units) * NJ
        nf = len(fillers)
        stride = max(1, total // max(nf, 1))
        fi = 0
        for t in range(total + LAG):
            if t < total:
                u, j = divmod(t, NJ)
                units[u].scores(j)
            if fi < nf and t % stride == stride - 1:
                fillers[fi]()
                fi += 1
            if t >= LAG:
                u, j = divmod(t - LAG, NJ)
                units[u].ctx_acc(j)
                if j == NJ - 1:
                    units[u].tail()
        while fi < nf:
            fillers[fi]()
            fi += 1

    # prefix: Q/K/V for dc 0 (V only in pass 0); pass dc p+1 interleaves
    # into attention pair p
    for th in qk_pass_thunks(0, with_v=True):
        th()
    for pair in range(DC):
        fillers = (qk_pass_thunks(pair + 1) if pair + 1 < DC else [])
        attn_pair(pair, fillers)


def _build():
    if "nc" in _CACHE:
        return _CACHE["nc"]
    nc = bacc.Bacc("TRN2", target_bir_lowering=False, debug=False,
                   num_devices=NCORES)
    names_shapes = [
        ("xt", [H, S], BF16), ("wq", [H, DG], BF16), ("wk", [H, DG], BF16),
        ("wv", [H, DG], BF16),
        ("bq", [128, DC], F32), ("bk", [128, DC], F32), ("bv", [1, DG], BF16),
        ("cos", [128, S], BF16), ("sin", [128, S], F32),
        ("ones", [128, IBLK], BF16), ("rsw", [128, 128], BF16),
    ]
    ins = [nc.dram_tensor(n, s, dt, kind="ExternalInput").ap()
           for n, s, dt in names_shapes]
    o_d = nc.dram_tensor("o", [DG, S], F32, kind="ExternalOutput").ap()
    from contextlib import ExitStack
    with tile.TileContext(nc) as tc:
        with ExitStack() as ctx:
            _emit(nc, tc, ctx, ins, o_d)
    nc.compile()
    _CACHE["nc"] = nc
    return nc


def _rotary_tables():
    inv_freq = (1.0 / (10000.0 ** (np.arange(0, HD, 2, dtype=np.float32)
                                   / np.float32(HD)))).astype(np.float32)
    t = np.arange(S, dtype=np.float32)
    freqs = np.outer(t, inv_freq).astype(np.float32)       # [S, 32]
    emb = np.concatenate([freqs, freqs], axis=-1)          # [S, 64]
    cos_t = np.cos(emb).T.astype(np.float32)               # [64, S]
    sin_t = np.sin(emb).T.astype(np.float32)               # unsigned
    cos2 = np.ascontiguousarray(np.concatenate([cos_t, cos_t], axis=0))
    sin2 = np.ascontiguousarray(np.concatenate([sin_t, sin_t], axis=0))
    # signed rotate-half permutation: t2_pre[d] = sign(d) * q[swap(d)],
    # sign = -1 on first half of each 64-block
    rsw = np.zeros((128, 128), dtype=np.float32)
    for d in range(128):
        blk, dd = d // 64, d % 64
        src = blk * 64 + (dd + 32) % 64
        rsw[src, d] = -1.0 if dd < 32 else 1.0
    return cos2, sin2, rsw


def _in_maps(hidden_states, Wq, bq, Wk, bk, Wv, bv):
    cos2, sin2, rsw = _rotary_tables()
    xts = [np.ascontiguousarray(hidden_states[b].T).astype(NPBF16)
           for b in range(B)]
    w_slices = {}
    for g in range(2):
        dsl = slice(g * DG, (g + 1) * DG)
        w_slices[g] = dict(
            wq=np.ascontiguousarray(Wq[:, dsl]).astype(NPBF16),
            wk=np.ascontiguousarray(Wk[:, dsl]).astype(NPBF16),
            wv=np.ascontiguousarray(Wv[:, dsl]).astype(NPBF16),
            bq=np.ascontiguousarray(bq[dsl].reshape(DC, 128).T),
            bk=np.ascontiguousarray(bk[dsl].reshape(DC, 128).T),
            bv=np.ascontiguousarray(bv[dsl].reshape(1, DG)).astype(NPBF16),
        )
    onesm = np.ones((128, IBLK), dtype=NPBF16)
    maps = []
    for c in range(NCORES):
        b, g = c // 2, c % 2
        m = {"xt": xts[b], "cos": cos2.astype(NPBF16), "sin": sin2,
             "ones": onesm, "rsw": rsw.astype(NPBF16)}
        m.update(w_slices[g])
        maps.append(m)
    return maps


def run(inputs, **kw):
    inputs = {k: np.asarray(v, dtype=np.float32) for k, v in inputs.items()}
    nc = _build()
    maps = _in_maps(**inputs)
    try:
        res = bass_utils.run_bass_kernel_spmd(
            nc, maps, core_ids=list(range(NCORES)), **kw)
    except Exception:
        # transient device errors (e.g. NRT_EXEC_UNIT_UNRECOVERABLE) clear on
        # retry
        res = bass_utils.run_bass_kernel_spmd(
            nc, maps, core_ids=list(range(NCORES)), **kw)
    out = np.empty((B, S, H), dtype=np.float32)
    for c in range(NCORES):
        b, g = c // 2, c % 2
        out[b, :, g * DG:(g + 1) * DG] = res.results[c]["o"].T
    return out, res


def kernel(**inputs):
    out, _ = run(inputs)
    return out
